# revision 34
# baseline (speedup 1.0000x reference)
"""WaveNet-style decoder (nn_DecoderV2) on 8 TRN2 NeuronCores.

Strategy: pure data parallel over batch (1024 -> 8 x 128). Per core the
recurrence runs with activations stored transposed [feature, batch] so the
batch lives on the free dim and every dense layer is a TensorE matmul with
stationary weights.

Layout:
  - hist[i] bf16 [128, T*128]: block t = [state_i(t) on partitions 0:64 ;
    IN_i(t) on partitions 64:128].  The dilated conv for layer i is a pair
    of K=128 matmuls with rhs = hist[i][:, t-block].
  - pd [64, 256] = [filter(128) | gate(128)] along the free dim, gate
    pre-scaled by 0.5 in the weights, so tanh(filter) and
    sigmoid(gate) = (tanh(gate/2)+1)/2 come from a SINGLE ACT op
    u = tanh(pd).  Then 2*gated = u_f * (u_g + 1) is one DVE
    scalar_tensor_tensor, and W4 is pre-scaled by 0.5 to compensate.
  - encoder states: only 55 of the 1008 (dil, pos) columns are ever read by
    the reference; sliced host-side and shipped pre-transposed.
  - Residual adds write straight into the hist tiles (they ARE the
    "appended" state of the reference); state slots for step t+1 are filled
    off the critical path by SBUF->SBUF DMA (t+1 >= d) or GpSimd copies
    (encoder region).

All matmul operands are bf16 (PSUM accumulation fp32); elementwise math
reads fp32 PSUM.  Biases in setup_inputs() are structurally zero; they are
still honored generally (ACT bias / DVE scalar operands) except b2, which
must be zero for the fused-gate trick (asserted at runtime).
"""

import numpy as np

import concourse.bacc as bacc
import concourse.mybir as mybir
import concourse.tile as tile
from concourse.bass_utils import run_bass_kernel_spmd

F32 = mybir.dt.float32
BF16 = mybir.dt.bfloat16

N_CORES = 8
B = 1024
BL = B // N_CORES          # 128 batch per core
T = 24
F = 64
HID = 128
DILATIONS = (1, 2, 4, 8, 16, 32)
L = 168
ENC_N = [min(d, T) for d in DILATIONS]
ENC_OFF = np.concatenate([[0], np.cumsum(ENC_N)]).astype(int).tolist()
ENC_TOT = int(np.sum(ENC_N))                     # 55

# wpackb (bf16) column layout.  "f|g" slots hold [cols 0:64 filter,
# 64:128 = 0.5*gate] so tanh(pd) yields [tanh(f), 2*sigmoid(g)-1].
_C_W1 = 0          # [16, 64]
_C_W2 = 64         # [64, 128]  f|g of W2      (rows 0:64)
_C_W3 = 192        # [64, 128]  f|g of W3      (rows 64:128!)
_C_M34 = 320       # [64, 128]  f|g of 0.5*W4r@W3  (rows 0:64)
_C_W4 = 448        # [64, 128]  0.5*W4         (rows 0:64)
_C_W5 = 576        # 3 x [128, 128] chunks     (rows 0:128)
_C_W6 = 960        # [128, 1]
_CB = 961

_CACHE = {}


def _bf16(a):
    """fp32 array -> ml_dtypes.bfloat16 (round to nearest even)."""
    import ml_dtypes
    return np.asarray(a, dtype=np.float32).astype(ml_dtypes.bfloat16)


def _build(b1, b4, b5, b6):
    nc = bacc.Bacc("TRN2", target_bir_lowering=False, debug=False,
                   num_devices=N_CORES)

    feat_in = nc.dram_tensor("featsrc", [16, T * BL], BF16,
                             kind="ExternalInput").ap()
    enc_in = nc.dram_tensor("encpack", [F, ENC_TOT * BL], BF16,
                            kind="ExternalInput").ap()
    wb_in = nc.dram_tensor("wpackb", [128, _CB], BF16,
                           kind="ExternalInput").ap()
    y_out = nc.dram_tensor("yout", [1, T * BL], F32,
                           kind="ExternalOutput").ap()

    AF = mybir.ActivationFunctionType
    OP = mybir.AluOpType
    b6f = float(b6[0])

    with tile.TileContext(nc) as tc:
        with (
            tc.tile_pool(name="const", bufs=1) as cp,
            tc.tile_pool(name="us_p", bufs=4) as us_p,
            tc.tile_pool(name="mg_p", bufs=4) as mg_p,
            tc.tile_pool(name="sk_p", bufs=2) as sk_p,
            tc.tile_pool(name="h_p", bufs=2) as h_p,
            tc.tile_pool(name="pd_p", bufs=5, space="PSUM") as pd_p,
            tc.tile_pool(name="po_p", bufs=2, space="PSUM") as po_p,
            tc.tile_pool(name="misc_p", bufs=1, space="PSUM") as misc_p,
        ):
            featT = cp.tile([16, T * BL], BF16, tag="featT")
            encsb = cp.tile([F, ENC_TOT * BL], BF16, tag="encsb")
            wb = cp.tile([128, _CB], BF16, tag="wb")
            bias = cp.tile([128, 4], F32, tag="bias")
            hist = [cp.tile([128, T * BL], BF16, tag=f"hist{i}",
                            name=f"hist{i}") for i in range(6)]
            yout_sb = cp.tile([1, T * BL], F32, tag="yout_sb")

            # bias tile built on-device from immediates (all tiny)
            nc.vector.memset(bias[:], 0.0)
            for col, vec, rows in ((0, b1, range(0, 64)),
                                   (1, b4[0:64], range(0, 64)),
                                   (2, b4[64:128], range(64, 128)),
                                   (3, b5, range(0, 128))):
                if float(np.abs(vec).max()) != 0.0:
                    for r0, v in zip(rows, np.asarray(vec, np.float32)):
                        nc.vector.memset(bias[r0:r0 + 1, col:col + 1],
                                         float(v))

            nc.sync.dma_start(featT[:], feat_in[:])
            nc.sync.dma_start(encsb[:], enc_in[:])
            nc.sync.dma_start(wb[:], wb_in[:])

            W1s = wb[0:16, _C_W1:_C_W1 + 64]
            W2f = wb[0:64, _C_W2:_C_W2 + 64]
            W2g = wb[0:64, _C_W2 + 64:_C_W2 + 128]
            W3f = wb[64:128, _C_W3:_C_W3 + 64]
            W3g = wb[64:128, _C_W3 + 64:_C_W3 + 128]
            M34f = wb[0:64, _C_M34:_C_M34 + 64]
            M34g = wb[0:64, _C_M34 + 64:_C_M34 + 128]
            W4s = wb[0:64, _C_W4:_C_W4 + 128]
            W5s = [wb[:, _C_W5 + c * 128:_C_W5 + (c + 1) * 128]
                   for c in range(3)]
            W6s = wb[:, _C_W6:_C_W6 + 1]
            b1s = bias[0:64, 0:1]
            b4lo = bias[0:64, 1:2]
            b4hi = bias[64:128, 2:3]
            b5s = bias[:, 3:4]

            def blk(t):
                return slice(t * BL, (t + 1) * BL)

            def state_copy(i, t):
                d = DILATIONS[i]
                dst = hist[i][0:64, blk(t)]
                if t >= d:
                    nc.sync.dma_start(dst, hist[i + 1][64:128, blk(t - d)])
                else:
                    nc.gpsimd.tensor_copy(dst,
                                          encsb[:, blk(ENC_OFF[i] + t)])

            for i in range(6):
                state_copy(i, 0)

            # HAM warm-up: ~3.5us of dense back-to-back matmuls into a
            # scratch PSUM tile flips the PE clock gate to 8/8 (2.4 GHz).
            # The kernel then never idles PE for a full 4096-cycle window,
            # so it stays warm.  Overlaps the input DMAs; results unused.
            wu = pd_p.tile([64, 2 * BL], F32, tag="pd", name="warmup")
            for w in range(20):
                nc.tensor.matmul(wu[:], wb[:, 0:64], wb[:, 0:2 * BL],
                                 start=(w == 0), stop=(w == 19))

            for t in range(T):
                # pd[k] accumulates (2-layer telescope):
                #   W2@state(k) + W3@IN[max(0,k-2)]
                #   + sum_{j=max(0,k-2)}^{k-1} M34@mg_j
                # One accumulation group per pd tile: the first matmul
                # (start=True) clears the whole bank; every later matmul
                # uses start=False (fresh regions are first-writes via the
                # per-element has_written bits, written regions accumulate).
                pds = [pd_p.tile([64, 2 * BL], F32, tag="pd",
                                 name=f"pd{t}_{i}") if i < 5 else None
                       for i in range(6)]
                for i in range(5):
                    nc.tensor.matmul(pds[i][:, 0:BL], W2f,
                                     hist[i][0:64, blk(t)],
                                     start=True, stop=False)
                for i in range(5):
                    nc.tensor.matmul(pds[i][:, BL:2 * BL], W2g,
                                     hist[i][0:64, blk(t)],
                                     start=False, stop=False)

                # -- chain head: IN_0(t) = tanh([y_{t-1}, feat_t]@W1 + b1)
                pin = misc_p.tile([64, BL], F32, tag="misc", name="pin")
                nc.tensor.matmul(pin[:], W1s, featT[:, blk(t)],
                                 start=True, stop=True)
                nc.scalar.activation(hist[0][64:128, blk(t)], pin[:],
                                     AF.Tanh, bias=b1s)
                IN0 = hist[0][64:128, blk(t)]
                # W3 @ IN_0 feeds pd[0] (chain), pd[1], pd[2]
                nc.tensor.matmul(pds[0][:, 0:BL], W3f, IN0,
                                 start=False, stop=False)
                nc.tensor.matmul(pds[0][:, BL:2 * BL], W3g, IN0,
                                 start=False, stop=True)
                for k in (1, 2):
                    nc.tensor.matmul(pds[k][:, 0:BL], W3f, IN0,
                                     start=False, stop=False)
                for k in (1, 2):
                    nc.tensor.matmul(pds[k][:, BL:2 * BL], W3g, IN0,
                                     start=False, stop=False)

                sk3 = sk_p.tile([128, 3 * BL], BF16, tag="sk3")
                ph = misc_p.tile([HID, BL], F32, tag="misc", name="ph")
                pos = [None] * 6
                mgs = [None] * 6

                def relu_skip(j):
                    # skip chunk j//2, partition half j%2 (ACT can shift)
                    r = (j % 2) * 64
                    c = (j // 2) * BL
                    nc.scalar.activation(sk3[r:r + 64, c:c + BL],
                                         pos[j][0:64, :], AF.Relu,
                                         bias=b4lo)

                for i in range(6):
                    # CHAIN: u -> mg (-> mg2) -> M34 pair into pd[i+1]
                    us = us_p.tile([64, 2 * BL], BF16, tag="us")
                    nc.scalar.activation(us[:], pds[i][:], AF.Tanh)
                    mg = mg_p.tile([64, BL], BF16, tag="mg")
                    nc.vector.scalar_tensor_tensor(
                        mg[:], us[:, BL:2 * BL], 1.0, us[:, 0:BL],
                        op0=OP.add, op1=OP.mult)
                    mgs[i] = mg
                    if i == 0:
                        # pd[5]'s bank frees up once u(0) has read pd[0]
                        pds[5] = pd_p.tile([64, 2 * BL], F32, tag="pd",
                                           name=f"pd{t}_5")
                        nc.tensor.matmul(pds[5][:, 0:BL], W2f,
                                         hist[5][0:64, blk(t)],
                                         start=True, stop=False)
                        nc.tensor.matmul(pds[5][:, BL:2 * BL], W2g,
                                         hist[5][0:64, blk(t)],
                                         start=False, stop=False)
                    if i < 5:
                        # chain pair: pd[i+1] += M34 @ mg_i ; off-chain
                        # second pair: pd[i+2] += M34 @ mg_i (f,f,g,g order
                        # shares each LDWEIGHTS between two matmuls)
                        nc.tensor.matmul(pds[i + 1][:, 0:BL], M34f, mg[:],
                                         start=False, stop=False)
                        if i < 4:
                            nc.tensor.matmul(pds[i + 2][:, 0:BL], M34f,
                                             mg[:], start=False, stop=False)
                        nc.tensor.matmul(pds[i + 1][:, BL:2 * BL], M34g,
                                         mg[:], start=False, stop=True)
                        if i < 4:
                            nc.tensor.matmul(pds[i + 2][:, BL:2 * BL],
                                             M34g, mg[:],
                                             start=False, stop=False)
                    po = po_p.tile([128, BL], F32, tag="po")
                    nc.tensor.matmul(po[:], W4s, mg[:], start=True,
                                     stop=True)
                    pos[i] = po
                    # deferred-by-one-layer work for layer j = i-1: the
                    # residual add, W3@IN[j+1] into pd[j+3], skip relu, W5.
                    if i >= 1:
                        j = i - 1
                        nc.vector.scalar_tensor_tensor(
                            hist[j + 1][64:128, blk(t)],
                            pos[j][64:128, :], b4hi,
                            hist[j][64:128, blk(t)],
                            op0=OP.add, op1=OP.add)
                        if j + 3 < 6:
                            INj1 = hist[j + 1][64:128, blk(t)]
                            nc.tensor.matmul(pds[j + 3][:, 0:BL], W3f,
                                             INj1, start=False, stop=False)
                            nc.tensor.matmul(pds[j + 3][:, BL:2 * BL],
                                             W3g, INj1,
                                             start=False, stop=False)
                        relu_skip(j)
                        if j % 2 == 1:
                            c = j // 2
                            nc.tensor.matmul(ph[:], W5s[c],
                                             sk3[:, c * BL:(c + 1) * BL],
                                             start=(c == 0), stop=False)

                # tail: layer 5's deferred work, then the head projection
                relu_skip(5)
                nc.tensor.matmul(ph[:], W5s[2], sk3[:, 2 * BL:3 * BL],
                                 start=False, stop=True)
                h = h_p.tile([HID, BL], BF16, tag="h")
                nc.vector.tensor_scalar(
                    out=h[:], in0=ph[:], scalar1=b5s, scalar2=0.0,
                    op0=OP.add, op1=OP.max)
                py = misc_p.tile([1, BL], F32, tag="misc", name="py")
                nc.tensor.matmul(py[:], W6s, h[:], start=True, stop=True)
                if t + 1 < T:
                    nc.vector.tensor_scalar(
                        out=featT[0:1, blk(t + 1)], in0=py[:], scalar1=b6f,
                        scalar2=None, op0=OP.add, op1=OP.bypass)
                    for i in range(6):
                        state_copy(i, t + 1)
                nc.scalar.activation(yout_sb[0:1, blk(t)], py[:],
                                     AF.Identity, bias=b6f)

            nc.sync.dma_start(y_out[:], yout_sb[:])

    nc.compile()
    return nc


def _pack_inputs(decoder_features, decoder_init_input, encoder_states,
                 W1, W2, W3, W4, W5, W6):
    """Host-side shard + transpose + bf16-cast.  Returns per-core in_maps."""
    wbp = np.zeros((128, _CB), np.float32)
    wbp[0:16, _C_W1:_C_W1 + 64] = W1
    wbp[0:64, _C_W2:_C_W2 + 64] = W2[:, 0:64]
    wbp[0:64, _C_W2 + 64:_C_W2 + 128] = 0.5 * W2[:, 64:128]
    wbp[64:128, _C_W3:_C_W3 + 64] = W3[:, 0:64]
    wbp[64:128, _C_W3 + 64:_C_W3 + 128] = 0.5 * W3[:, 64:128]
    # M34: residual-into-next-dilated shortcut, rhs is mg = 2*gated
    M34 = 0.5 * (W4[:, 64:128] @ W3)                 # [64, 128]
    wbp[0:64, _C_M34:_C_M34 + 64] = M34[:, 0:64]
    wbp[0:64, _C_M34 + 64:_C_M34 + 128] = 0.5 * M34[:, 64:128]
    wbp[0:64, _C_W4:_C_W4 + 128] = 0.5 * W4
    for c in range(3):
        wbp[:, _C_W5 + c * 128:_C_W5 + (c + 1) * 128] = \
            W5[c * 128:(c + 1) * 128, :]
    wbp[:, _C_W6:_C_W6 + 1] = W6
    wb_bits = _bf16(wbp)

    in_maps = []
    for c in range(N_CORES):
        s = slice(c * BL, (c + 1) * BL)
        # featT [16, T*BL]: row 0 block 0 = init, rows 1:16 = features^T
        ft = np.zeros((16, T, BL), np.float32)
        ft[0, 0, :] = decoder_init_input[s, 0]
        ft[1:16] = decoder_features[s].transpose(2, 1, 0)
        # encpack [64, ENC_TOT*BL]
        ep = np.zeros((F, ENC_TOT, BL), np.float32)
        for i, d in enumerate(DILATIONS):
            n = ENC_N[i]
            ep[:, ENC_OFF[i]:ENC_OFF[i] + n, :] = \
                encoder_states[i, s, L - d:L - d + n, :].transpose(2, 1, 0)
        in_maps.append({
            "featsrc": _bf16(ft.reshape(16, T * BL)),
            "encpack": _bf16(ep.reshape(F, ENC_TOT * BL)),
            "wpackb": wb_bits,
        })
    return in_maps


def kernel(**inputs):
    decoder_features = np.asarray(inputs["decoder_features"], np.float32)
    decoder_init_input = np.asarray(inputs["decoder_init_input"], np.float32)
    encoder_states = np.asarray(inputs["encoder_states"], np.float32)
    W1 = np.asarray(inputs["W1"], np.float32)
    b1 = np.asarray(inputs["b1"], np.float32)
    W2 = np.asarray(inputs["W2"], np.float32)
    b2 = np.asarray(inputs["b2"], np.float32)
    W3 = np.asarray(inputs["W3"], np.float32)
    W4 = np.asarray(inputs["W4"], np.float32)
    b4 = np.asarray(inputs["b4"], np.float32)
    W5 = np.asarray(inputs["W5"], np.float32)
    b5 = np.asarray(inputs["b5"], np.float32)
    W6 = np.asarray(inputs["W6"], np.float32)
    b6 = np.asarray(inputs["b6"], np.float32)
    assert float(np.abs(b2).max()) == 0.0, "kernel assumes b2 == 0"
    assert float(np.abs(b4).max()) == 0.0, \
        "kernel's telescoped dilated accumulation assumes b4 == 0"

    key = "nc"
    if key not in _CACHE:
        _CACHE[key] = _build(b1, b4, b5, b6)
    nc = _CACHE[key]

    in_maps = _pack_inputs(decoder_features, decoder_init_input,
                           encoder_states, W1, W2, W3, W4, W5, W6)
    res = run_bass_kernel_spmd(nc, in_maps, list(range(N_CORES)))

    out = np.empty((B, T, 1), np.float32)
    for c in range(N_CORES):
        y = res.results[c]["yout"].reshape(T, BL)
        out[c * BL:(c + 1) * BL, :, 0] = y.T
    return out


# revision 35
# speedup vs baseline: 1.0182x; 1.0182x over previous
"""WaveNet-style decoder (nn_DecoderV2) on 8 TRN2 NeuronCores.

Strategy: pure data parallel over batch (1024 -> 8 x 128). Per core the
recurrence runs with activations stored transposed [feature, batch] so the
batch lives on the free dim and every dense layer is a TensorE matmul with
stationary weights.

Layout:
  - hist[i] bf16 [128, T*128]: block t = [state_i(t) on partitions 0:64 ;
    IN_i(t) on partitions 64:128].  The dilated conv for layer i is a pair
    of K=128 matmuls with rhs = hist[i][:, t-block].
  - pd [64, 256] = [filter(128) | gate(128)] along the free dim, gate
    pre-scaled by 0.5 in the weights, so tanh(filter) and
    sigmoid(gate) = (tanh(gate/2)+1)/2 come from a SINGLE ACT op
    u = tanh(pd).  Then 2*gated = u_f * (u_g + 1) is one DVE
    scalar_tensor_tensor, and W4 is pre-scaled by 0.5 to compensate.
  - encoder states: only 55 of the 1008 (dil, pos) columns are ever read by
    the reference; sliced host-side and shipped pre-transposed.
  - Residual adds write straight into the hist tiles (they ARE the
    "appended" state of the reference); state slots for step t+1 are filled
    off the critical path by SBUF->SBUF DMA (t+1 >= d) or GpSimd copies
    (encoder region).

All matmul operands are bf16 (PSUM accumulation fp32); elementwise math
reads fp32 PSUM.  Biases in setup_inputs() are structurally zero; they are
still honored generally (ACT bias / DVE scalar operands) except b2, which
must be zero for the fused-gate trick (asserted at runtime).
"""

import numpy as np

import concourse.bacc as bacc
import concourse.mybir as mybir
import concourse.tile as tile
from concourse.bass_utils import run_bass_kernel_spmd

F32 = mybir.dt.float32
BF16 = mybir.dt.bfloat16

N_CORES = 8
B = 1024
BL = B // N_CORES          # 128 batch per core
T = 24
F = 64
HID = 128
DILATIONS = (1, 2, 4, 8, 16, 32)
L = 168
ENC_N = [min(d, T) for d in DILATIONS]
ENC_OFF = np.concatenate([[0], np.cumsum(ENC_N)]).astype(int).tolist()
ENC_TOT = int(np.sum(ENC_N))                     # 55

# wpackb (bf16) column layout.  "f|g" slots hold [cols 0:64 filter,
# 64:128 = 0.5*gate] so tanh(pd) yields [tanh(f), 2*sigmoid(g)-1].
_C_W1 = 0          # [16, 64]
_C_W2 = 64         # [64, 128]  f|g of W2      (rows 0:64)
_C_W3 = 192        # [64, 128]  f|g of W3      (rows 64:128!)
_C_M34 = 320       # [64, 128]  f|g of 0.5*W4r@W3  (rows 0:64)
_C_W4 = 448        # [64, 128]  0.5*W4         (rows 0:64)
_C_W5 = 576        # 3 x [128, 128] chunks     (rows 0:128)
_C_W6 = 960        # [128, 1]
_CB = 961

_CACHE = {}


def _bf16(a):
    """fp32 array -> ml_dtypes.bfloat16 (round to nearest even)."""
    import ml_dtypes
    return np.asarray(a, dtype=np.float32).astype(ml_dtypes.bfloat16)


def _build(b1, b4, b5, b6):
    nc = bacc.Bacc("TRN2", target_bir_lowering=False, debug=False,
                   num_devices=N_CORES)

    feat_in = nc.dram_tensor("featsrc", [16, T * BL], BF16,
                             kind="ExternalInput").ap()
    enc_in = nc.dram_tensor("encpack", [F, ENC_TOT * BL], BF16,
                            kind="ExternalInput").ap()
    wb_in = nc.dram_tensor("wpackb", [128, _CB], BF16,
                           kind="ExternalInput").ap()
    y_out = nc.dram_tensor("yout", [1, T * BL], F32,
                           kind="ExternalOutput").ap()

    AF = mybir.ActivationFunctionType
    OP = mybir.AluOpType
    b6f = float(b6[0])

    with tile.TileContext(nc) as tc:
        with (
            tc.tile_pool(name="const", bufs=1) as cp,
            tc.tile_pool(name="us_p", bufs=4) as us_p,
            tc.tile_pool(name="mg_p", bufs=4) as mg_p,
            tc.tile_pool(name="sk_p", bufs=2) as sk_p,
            tc.tile_pool(name="h_p", bufs=2) as h_p,
            tc.tile_pool(name="pd_p", bufs=5, space="PSUM") as pd_p,
            tc.tile_pool(name="po_p", bufs=2, space="PSUM") as po_p,
            tc.tile_pool(name="misc_p", bufs=1, space="PSUM") as misc_p,
        ):
            featT = cp.tile([16, T * BL], BF16, tag="featT")
            encsb = cp.tile([F, ENC_TOT * BL], BF16, tag="encsb")
            wb = cp.tile([128, _CB], BF16, tag="wb")
            bias = cp.tile([128, 4], F32, tag="bias")
            hist = [cp.tile([128, T * BL], BF16, tag=f"hist{i}",
                            name=f"hist{i}") for i in range(6)]
            yout_sb = cp.tile([1, T * BL], F32, tag="yout_sb")

            # bias tile built on-device from immediates (all tiny)
            nc.vector.memset(bias[:], 0.0)
            for col, vec, rows in ((0, b1, range(0, 64)),
                                   (1, b4[0:64], range(0, 64)),
                                   (2, b4[64:128], range(64, 128)),
                                   (3, b5, range(0, 128))):
                if float(np.abs(vec).max()) != 0.0:
                    for r0, v in zip(rows, np.asarray(vec, np.float32)):
                        nc.vector.memset(bias[r0:r0 + 1, col:col + 1],
                                         float(v))

            nc.sync.dma_start(featT[:], feat_in[:])
            nc.sync.dma_start(encsb[:], enc_in[:])
            nc.sync.dma_start(wb[:], wb_in[:])

            W1s = wb[0:16, _C_W1:_C_W1 + 64]
            W2f = wb[0:64, _C_W2:_C_W2 + 64]
            W2g = wb[0:64, _C_W2 + 64:_C_W2 + 128]
            W3f = wb[64:128, _C_W3:_C_W3 + 64]
            W3g = wb[64:128, _C_W3 + 64:_C_W3 + 128]
            M34f = wb[0:64, _C_M34:_C_M34 + 64]
            M34g = wb[0:64, _C_M34 + 64:_C_M34 + 128]
            W4s = wb[0:64, _C_W4:_C_W4 + 128]
            W5s = [wb[:, _C_W5 + c * 128:_C_W5 + (c + 1) * 128]
                   for c in range(3)]
            W6s = wb[:, _C_W6:_C_W6 + 1]
            b1s = bias[0:64, 0:1]
            b4lo = bias[0:64, 1:2]
            b4hi = bias[64:128, 2:3]
            b5s = bias[:, 3:4]

            def blk(t):
                return slice(t * BL, (t + 1) * BL)

            def state_copy(i, t):
                d = DILATIONS[i]
                dst = hist[i][0:64, blk(t)]
                if t >= d:
                    nc.sync.dma_start(dst, hist[i + 1][64:128, blk(t - d)])
                else:
                    nc.gpsimd.tensor_copy(dst,
                                          encsb[:, blk(ENC_OFF[i] + t)])

            for i in range(6):
                state_copy(i, 0)

            # HAM warm-up: ~3.5us of dense back-to-back matmuls into a
            # scratch PSUM tile flips the PE clock gate to 8/8 (2.4 GHz).
            # The kernel then never idles PE for a full 4096-cycle window,
            # so it stays warm.  Overlaps the input DMAs; results unused.
            wu = pd_p.tile([64, 2 * BL], F32, tag="pd", name="warmup")
            for w in range(20):
                nc.tensor.matmul(wu[:], wb[:, 0:64], wb[:, 0:2 * BL],
                                 start=(w == 0), stop=(w == 19))

            for t in range(T):
                # pd[k] accumulates (2-layer telescope):
                #   W2@state(k) + W3@IN[max(0,k-2)]
                #   + sum_{j=max(0,k-2)}^{k-1} M34@mg_j
                # One accumulation group per pd tile: the first matmul
                # (start=True) clears the whole bank; every later matmul
                # uses start=False (fresh regions are first-writes via the
                # per-element has_written bits, written regions accumulate).
                pds = [pd_p.tile([64, 2 * BL], F32, tag="pd",
                                 name=f"pd{t}_{i}") if i < 5 else None
                       for i in range(6)]
                for i in range(5):
                    nc.tensor.matmul(pds[i][:, 0:BL], W2f,
                                     hist[i][0:64, blk(t)],
                                     start=True, stop=False)
                for i in range(5):
                    nc.tensor.matmul(pds[i][:, BL:2 * BL], W2g,
                                     hist[i][0:64, blk(t)],
                                     start=False, stop=False)

                # -- chain head: IN_0(t) = tanh([y_{t-1}, feat_t]@W1 + b1)
                pin = misc_p.tile([64, BL], F32, tag="misc", name="pin")
                nc.tensor.matmul(pin[:], W1s, featT[:, blk(t)],
                                 start=True, stop=True)
                nc.scalar.activation(hist[0][64:128, blk(t)], pin[:],
                                     AF.Tanh, bias=b1s)
                IN0 = hist[0][64:128, blk(t)]
                # W3 @ IN_0 feeds pd[0] (chain), pd[1], pd[2]
                nc.tensor.matmul(pds[0][:, 0:BL], W3f, IN0,
                                 start=False, stop=False)
                nc.tensor.matmul(pds[0][:, BL:2 * BL], W3g, IN0,
                                 start=False, stop=True)
                for k in (1, 2):
                    nc.tensor.matmul(pds[k][:, 0:BL], W3f, IN0,
                                     start=False, stop=False)
                for k in (1, 2):
                    nc.tensor.matmul(pds[k][:, BL:2 * BL], W3g, IN0,
                                     start=False, stop=False)

                sk3 = sk_p.tile([128, 3 * BL], BF16, tag="sk3")
                ph = misc_p.tile([HID, BL], F32, tag="misc", name="ph")
                pos = [None] * 6
                mgs = [None] * 6

                def relu_skip(j):
                    # skip chunk j//2, partition half j%2 (ACT can shift)
                    r = (j % 2) * 64
                    c = (j // 2) * BL
                    nc.scalar.activation(sk3[r:r + 64, c:c + BL],
                                         pos[j][0:64, :], AF.Relu,
                                         bias=b4lo)

                for i in range(6):
                    # CHAIN: u -> mg (-> mg2) -> M34 pair into pd[i+1]
                    us = us_p.tile([64, 2 * BL], BF16, tag="us")
                    nc.scalar.activation(us[:], pds[i][:], AF.Tanh)
                    mg = mg_p.tile([64, BL], BF16, tag="mg")
                    nc.vector.scalar_tensor_tensor(
                        mg[:], us[:, BL:2 * BL], 1.0, us[:, 0:BL],
                        op0=OP.add, op1=OP.mult)
                    mgs[i] = mg
                    if i == 0:
                        # pd[5]'s bank frees up once u(0) has read pd[0]
                        pds[5] = pd_p.tile([64, 2 * BL], F32, tag="pd",
                                           name=f"pd{t}_5")
                        nc.tensor.matmul(pds[5][:, 0:BL], W2f,
                                         hist[5][0:64, blk(t)],
                                         start=True, stop=False)
                        nc.tensor.matmul(pds[5][:, BL:2 * BL], W2g,
                                         hist[5][0:64, blk(t)],
                                         start=False, stop=False)
                    if i < 5:
                        # chain pair FIRST (u(i+1) waits on it); the
                        # off-chain second pair re-loads weights but PE has
                        # slack and the chain gains ~110ns/layer.
                        nc.tensor.matmul(pds[i + 1][:, 0:BL], M34f, mg[:],
                                         start=False, stop=False)
                        nc.tensor.matmul(pds[i + 1][:, BL:2 * BL], M34g,
                                         mg[:], start=False, stop=True)
                        if i < 4:
                            nc.tensor.matmul(pds[i + 2][:, 0:BL], M34f,
                                             mg[:], start=False, stop=False)
                            nc.tensor.matmul(pds[i + 2][:, BL:2 * BL],
                                             M34g, mg[:],
                                             start=False, stop=False)
                    po = po_p.tile([128, BL], F32, tag="po")
                    nc.tensor.matmul(po[:], W4s, mg[:], start=True,
                                     stop=True)
                    pos[i] = po
                    # deferred-by-one-layer work for layer j = i-1: the
                    # residual add, W3@IN[j+1] into pd[j+3], skip relu, W5.
                    if i >= 1:
                        j = i - 1
                        nc.vector.scalar_tensor_tensor(
                            hist[j + 1][64:128, blk(t)],
                            pos[j][64:128, :], b4hi,
                            hist[j][64:128, blk(t)],
                            op0=OP.add, op1=OP.add)
                        if j + 3 < 6:
                            INj1 = hist[j + 1][64:128, blk(t)]
                            nc.tensor.matmul(pds[j + 3][:, 0:BL], W3f,
                                             INj1, start=False, stop=False)
                            nc.tensor.matmul(pds[j + 3][:, BL:2 * BL],
                                             W3g, INj1,
                                             start=False, stop=False)
                        relu_skip(j)
                        if j % 2 == 1:
                            c = j // 2
                            nc.tensor.matmul(ph[:], W5s[c],
                                             sk3[:, c * BL:(c + 1) * BL],
                                             start=(c == 0), stop=False)

                # tail: layer 5's deferred work, then the head projection
                relu_skip(5)
                nc.tensor.matmul(ph[:], W5s[2], sk3[:, 2 * BL:3 * BL],
                                 start=False, stop=True)
                h = h_p.tile([HID, BL], BF16, tag="h")
                nc.vector.tensor_scalar(
                    out=h[:], in0=ph[:], scalar1=b5s, scalar2=0.0,
                    op0=OP.add, op1=OP.max)
                py = misc_p.tile([1, BL], F32, tag="misc", name="py")
                nc.tensor.matmul(py[:], W6s, h[:], start=True, stop=True)
                if t + 1 < T:
                    nc.vector.tensor_scalar(
                        out=featT[0:1, blk(t + 1)], in0=py[:], scalar1=b6f,
                        scalar2=None, op0=OP.add, op1=OP.bypass)
                    for i in range(6):
                        state_copy(i, t + 1)
                nc.scalar.activation(yout_sb[0:1, blk(t)], py[:],
                                     AF.Identity, bias=b6f)

            nc.sync.dma_start(y_out[:], yout_sb[:])

    nc.compile()
    return nc


def _pack_inputs(decoder_features, decoder_init_input, encoder_states,
                 W1, W2, W3, W4, W5, W6):
    """Host-side shard + transpose + bf16-cast.  Returns per-core in_maps."""
    wbp = np.zeros((128, _CB), np.float32)
    wbp[0:16, _C_W1:_C_W1 + 64] = W1
    wbp[0:64, _C_W2:_C_W2 + 64] = W2[:, 0:64]
    wbp[0:64, _C_W2 + 64:_C_W2 + 128] = 0.5 * W2[:, 64:128]
    wbp[64:128, _C_W3:_C_W3 + 64] = W3[:, 0:64]
    wbp[64:128, _C_W3 + 64:_C_W3 + 128] = 0.5 * W3[:, 64:128]
    # M34: residual-into-next-dilated shortcut, rhs is mg = 2*gated
    M34 = 0.5 * (W4[:, 64:128] @ W3)                 # [64, 128]
    wbp[0:64, _C_M34:_C_M34 + 64] = M34[:, 0:64]
    wbp[0:64, _C_M34 + 64:_C_M34 + 128] = 0.5 * M34[:, 64:128]
    wbp[0:64, _C_W4:_C_W4 + 128] = 0.5 * W4
    for c in range(3):
        wbp[:, _C_W5 + c * 128:_C_W5 + (c + 1) * 128] = \
            W5[c * 128:(c + 1) * 128, :]
    wbp[:, _C_W6:_C_W6 + 1] = W6
    wb_bits = _bf16(wbp)

    in_maps = []
    for c in range(N_CORES):
        s = slice(c * BL, (c + 1) * BL)
        # featT [16, T*BL]: row 0 block 0 = init, rows 1:16 = features^T
        ft = np.zeros((16, T, BL), np.float32)
        ft[0, 0, :] = decoder_init_input[s, 0]
        ft[1:16] = decoder_features[s].transpose(2, 1, 0)
        # encpack [64, ENC_TOT*BL]
        ep = np.zeros((F, ENC_TOT, BL), np.float32)
        for i, d in enumerate(DILATIONS):
            n = ENC_N[i]
            ep[:, ENC_OFF[i]:ENC_OFF[i] + n, :] = \
                encoder_states[i, s, L - d:L - d + n, :].transpose(2, 1, 0)
        in_maps.append({
            "featsrc": _bf16(ft.reshape(16, T * BL)),
            "encpack": _bf16(ep.reshape(F, ENC_TOT * BL)),
            "wpackb": wb_bits,
        })
    return in_maps


def kernel(**inputs):
    decoder_features = np.asarray(inputs["decoder_features"], np.float32)
    decoder_init_input = np.asarray(inputs["decoder_init_input"], np.float32)
    encoder_states = np.asarray(inputs["encoder_states"], np.float32)
    W1 = np.asarray(inputs["W1"], np.float32)
    b1 = np.asarray(inputs["b1"], np.float32)
    W2 = np.asarray(inputs["W2"], np.float32)
    b2 = np.asarray(inputs["b2"], np.float32)
    W3 = np.asarray(inputs["W3"], np.float32)
    W4 = np.asarray(inputs["W4"], np.float32)
    b4 = np.asarray(inputs["b4"], np.float32)
    W5 = np.asarray(inputs["W5"], np.float32)
    b5 = np.asarray(inputs["b5"], np.float32)
    W6 = np.asarray(inputs["W6"], np.float32)
    b6 = np.asarray(inputs["b6"], np.float32)
    assert float(np.abs(b2).max()) == 0.0, "kernel assumes b2 == 0"
    assert float(np.abs(b4).max()) == 0.0, \
        "kernel's telescoped dilated accumulation assumes b4 == 0"

    key = "nc"
    if key not in _CACHE:
        _CACHE[key] = _build(b1, b4, b5, b6)
    nc = _CACHE[key]

    in_maps = _pack_inputs(decoder_features, decoder_init_input,
                           encoder_states, W1, W2, W3, W4, W5, W6)
    res = run_bass_kernel_spmd(nc, in_maps, list(range(N_CORES)))

    out = np.empty((B, T, 1), np.float32)
    for c in range(N_CORES):
        y = res.results[c]["yout"].reshape(T, BL)
        out[c * BL:(c + 1) * BL, :, 0] = y.T
    return out
